# revision 25
# baseline (speedup 1.0000x reference)
"""Trainium2 Bass kernel for nn_MemoryReader.

Reference computation (per batch b):
    mi = mk.reshape(CK, N);  qi = qk.reshape(CK, P) / sqrt(CK)
    S  = mi.T @ qi                      # [N, P] affinity logits
    A  = softmax(S, axis=0)             # over memory axis N
    mem = mv.reshape(CV, N) @ A         # [CV, P]
    out = concat([mem, qv], axis=channel)

Sharding: 8 cores = (4 batches) x (2 halves of the memory axis N).
Each core computes, for its (b, half):
    E      = exp(0.125*S - 2)                   # fp8e4; the -2 bias keeps
                                                # E<=54 (TRN fp8e4 max 240)
                                                # and cancels in the softmax
    memT   = E.T @ [mvT | 1]                    # [P, 513]: col 512 = sum(E)
The host combines: mem = (num_0 + num_1) / (den_0 + den_1), then concats
qv (pure passthrough). No on-device collectives needed.

Key speed features vs the bf16 version:
  - mm2 runs in fp8e4 DoubleRow mode: each matmul contracts TWO 128-row
    n-tiles (lhsT/rhs get [K, 2, M] APs), ~2x column throughput.
  - The softmax denominator is a 513th "ones" column of mvT, accumulated
    by the same mm2 matmuls (split 256+257 to satisfy the one-PSUM-bank
    rule, sharing one LDWEIGHTS via a dedupe pass) - no vector-engine
    accumulation at all.
  - exp() is fused 4 n-tiles per ACT instruction (PSUM "squad" tiles) to
    amortize the ~185ns per-instruction access latency.
"""

import numpy as np
import ml_dtypes

import concourse.tile as tile
from concourse import bacc, mybir
from concourse.bass_utils import run_bass_kernel_spmd

# Problem shape (hardcoded per contract)
B, CK, CV, T, H, W = 4, 64, 512, 8, 30, 54
N = T * H * W          # 12960 memory positions
P = H * W              # 1620 query positions
NHALF = N // 2         # 6480 per core
NT = (NHALF + 127) // 128   # 51 n-tiles (last has 80 rows)
NLAST = NHALF - (NT - 1) * 128  # 80
NTP = NT + 1           # pad to even tile count for DoubleRow pairing
MVW = 528              # mvT free width: 512 mv + 1 ones + 15 pad (16B align)
CVA = 513              # real mm2 output width (512 mv + 1 denominator)
ASPL = 256             # a-half columns (b-half = 257); each fits a PSUM bank
EXP_BIAS = -2.0        # exp(0.125*s - 2): range safety for fp8e4

# p-axis chunks of 256 (2 slices of 128 each; 84-wide remainder last — it
# has the smallest output-DMA tail. Running it first was tried and is 8%
# WORSE: its mm2 consumes one mvT tile per ~330ns, 2x what the DMA queues
# can sustain while the full mvT load is still in flight.)
PCH = [(0, 256), (256, 256), (512, 256), (768, 256), (1024, 256),
       (1280, 256), (1536, 84)]
QUADS = [(0, 4), (4, 4), (8, 4), (12, 4), (16, 4), (20, 4), (24, 4), (28, 4),
         (32, 4), (36, 4), (40, 4), (44, 4), (48, 3)]

DEDUPE_LDW = True

_CACHE = {}


def _build_program():
    f8 = mybir.dt.float8e4
    bf16 = mybir.dt.bfloat16
    f32 = mybir.dt.float32
    DR = mybir.MatmulPerfMode.DoubleRow
    nc = bacc.Bacc(None, target_bir_lowering=False, debug=False)

    # mk/qk zero-padded to K=128 on the host: full-row LDWEIGHTS go through
    # the background weight buffer (K=64 loads serialize on the weight port).
    # fp8 halves the DMA feed and doubles mm1's FWL weight-load rate; the
    # logit quantization error (~0.05 absolute) is well inside the margin.
    mk_d = nc.declare_dram_parameter("mk", [128, NT, 128], f8, isOutput=False)
    # padded to 2048 so the one qk transfer has 2KB per-partition rows
    # (sub-2KB rows run ~3x slower on the DMA path)
    qk_d = nc.declare_dram_parameter("qk", [128, 2048], f8, isOutput=False)
    mvt_d = nc.declare_dram_parameter("mvT", [128, NTP, MVW], f8, isOutput=False)
    mem_d = nc.declare_dram_parameter("memT", [P, CVA], f32, isOutput=True)

    with tile.TileContext(nc) as tc:
        with (
            tc.tile_pool(name="singles", bufs=1) as singles,
            tc.tile_pool(name="epool", bufs=3) as epool,
            tc.tile_pool(name="opool", bufs=4) as opool,
            tc.tile_pool(name="spsum", bufs=2, space="PSUM") as spsum,
            tc.tile_pool(name="apsum", bufs=2, space="PSUM") as apsum,
            tc.tile_pool(name="bpsum", bufs=2, space="PSUM") as bpsum,
        ):
            qk_sb = singles.tile([128, 2048], f8)
            mk_sb = singles.tile([128, NT, 128], f8)
            mvt_sb = singles.tile([128, NTP, MVW], f8)
            # DMA facts (measured): every transfer stripes over all 16
            # queues and completes in ISSUE ORDER at aggregate BW, but
            # sub-2KB per-partition rows run ~5x slower (per-row descriptor
            # overhead), and each trigger costs ~660ns on the Sync queue.
            # So: few, fat, consumption-ordered transfers.
            nc.sync.dma_start(out=qk_sb[:, :], in_=qk_d[:, :])
            mk_groups = [(0, 16), (16, 32), (32, NT)]
            mv_groups = [(g, min(g + 4, NTP)) for g in range(0, NTP, 4)]
            order = [("mk", 0), ("mv", 0), ("mv", 1), ("mv", 2), ("mk", 1),
                     ("mv", 3), ("mv", 4), ("mv", 5), ("mv", 6), ("mk", 2),
                     ("mv", 7), ("mv", 8), ("mv", 9), ("mv", 10), ("mv", 11),
                     ("mv", 12)]
            for kind, gi in order:
                if kind == "mk":
                    g0, g1 = mk_groups[gi]
                    nc.sync.dma_start(out=mk_sb[:, g0:g1, :],
                                      in_=mk_d[:, g0:g1, :])
                else:
                    g0, g1 = mv_groups[gi]
                    nc.sync.dma_start(out=mvt_sb[:, g0:g1, :],
                                      in_=mvt_d[:, g0:g1, :])

            # Warm-up: full-size matmuls on a memset tile, depending on no
            # DMA. They fill the initial PE idle gap AND release the HAM
            # clock throttle (~3.4us of sustained activity needed).
            warmw = singles.tile([128, 128], bf16, name="warmw")
            nc.vector.memset(warmw, 1.0)
            bias_sb = singles.tile([128, 1], f32, name="bias")
            nc.vector.memset(bias_sb, EXP_BIAS)
            # 24 bridges until the first inputs land (~5us); the real
            # matmul stream continues the HAM busy window.
            warm = spsum.tile([128, 128], f32, tag="s", name="warm")
            for _ in range(24):
                nc.tensor.matmul(warm, lhsT=warmw, rhs=warmw,
                                 start=True, stop=True)

            # Flat software pipeline over (chunk, quad) units: issue unit
            # u+1's mm1 before unit u's mm2 so the PE queue never stalls on
            # the ACT->mm2 dependency.
            units = [(ci, qi) for ci in range(len(PCH)) for qi in range(len(QUADS))]
            squads = {}
            e4s = {}
            accs = {}

            def issue_mm1(u):
                ci, qi = units[u]
                ps, w = PCH[ci]
                q0, qn = QUADS[qi]
                s = spsum.tile([128, 4, ASPL], f32, tag="s", name="s")
                for j in range(qn):
                    nt = q0 + j
                    nsz = 128 if nt < NT - 1 else NLAST
                    nc.tensor.matmul(
                        s[:nsz, j, :w],
                        lhsT=mk_sb[:, nt, :nsz],
                        rhs=qk_sb[:, ps:ps + w],
                        start=True,
                        stop=True,
                    )
                squads[u] = s

            def issue_act(u):
                ci, qi = units[u]
                ps, w = PCH[ci]
                q0, qn = QUADS[qi]
                s = squads.pop(u)
                e4 = epool.tile([128, 4, ASPL], f8, tag="e", name="e")
                # rows 80:128 of the tri-quad's last tile hold stale PSUM;
                # exp of garbage lands in e4 rows the matmuls never read.
                nc.scalar.activation(
                    out=e4[:, 0:qn, :w],
                    in_=s[:, 0:qn, :w],
                    func=mybir.ActivationFunctionType.Exp,
                    scale=0.125,  # 1/sqrt(CK)
                    bias=bias_sb[:, :],
                )
                e4s[u] = e4

            def issue_mm2(u):
                ci, qi = units[u]
                ps, w = PCH[ci]
                q0, qn = QUADS[qi]
                e4 = e4s.pop(u)
                nslices = (w + 127) // 128
                if qi == 0:
                    accs[ci] = [
                        (apsum.tile([128, 512], f32, tag="acc_a", name="acc_a"),
                         bpsum.tile([128, 512], f32, tag="acc_b", name="acc_b"))
                        for _ in range(nslices)
                    ]
                first = qi == 0
                last = qi == len(QUADS) - 1
                # DoubleRow pairs (and the odd single tile 50 at quad end)
                steps = []
                if qn == 4:
                    steps = [(0, True), (2, True)]
                else:
                    steps = [(0, True), (2, False)]
                for si, (j, dr) in enumerate(steps):
                    nt = q0 + j
                    st = first and si == 0
                    sp = last and si == len(steps) - 1
                    nsz = 128 if dr else NLAST
                    for sl in range(nslices):
                        pw = min(128, w - 128 * sl)
                        acc_a, acc_b = accs[ci][sl]
                        if dr:
                            el = e4[:nsz, j:j + 2, sl * 128:sl * 128 + pw]
                            nc.tensor.matmul(
                                acc_a[:pw, 0:ASPL],
                                lhsT=el,
                                rhs=mvt_sb[:nsz, nt:nt + 2, 0:ASPL],
                                start=st, stop=sp,
                                perf_mode=DR,
                            )
                            nc.tensor.matmul(
                                acc_b[:pw, 0:CVA - ASPL],
                                lhsT=el,
                                rhs=mvt_sb[:nsz, nt:nt + 2, ASPL:CVA],
                                start=st, stop=sp,
                                perf_mode=DR,
                            )
                        else:
                            el = e4[:nsz, j, sl * 128:sl * 128 + pw]
                            nc.tensor.matmul(
                                acc_a[:pw, 0:ASPL],
                                lhsT=el,
                                rhs=mvt_sb[:nsz, nt, 0:ASPL],
                                start=st, stop=sp,
                            )
                            nc.tensor.matmul(
                                acc_b[:pw, 0:CVA - ASPL],
                                lhsT=el,
                                rhs=mvt_sb[:nsz, nt, ASPL:CVA],
                                start=st, stop=sp,
                            )
                if last:
                    for sl in range(nslices):
                        pw = min(128, w - 128 * sl)
                        acc_a, acc_b = accs[ci][sl]
                        o_sb = opool.tile([128, CVA], f32, tag="o", name="o")
                        # a on DVE, b on ACT (idle at chunk boundaries):
                        # parallel copies halve the acc-slot turnaround that
                        # gates the next chunk's first start=True matmuls
                        nc.vector.tensor_copy(out=o_sb[:pw, 0:ASPL],
                                              in_=acc_a[:pw, 0:ASPL])
                        nc.scalar.activation(
                            out=o_sb[:pw, ASPL:CVA],
                            in_=acc_b[:pw, 0:CVA - ASPL],
                            func=mybir.ActivationFunctionType.Copy,
                        )
                        p0 = ps + sl * 128
                        nc.sync.dma_start(out=mem_d[p0:p0 + pw, :],
                                          in_=o_sb[:pw, :])
                    del accs[ci]

            issue_mm1(0)
            for u in range(len(units)):
                if u + 1 < len(units):
                    issue_mm1(u + 1)
                issue_act(u)
                issue_mm2(u)

    _strip_same_engine_waits(nc)
    if DEDUPE_LDW:
        _dedupe_ldweights(nc)
    nc.compile()
    return nc


def _ldw_key(inst):
    ap = inst.ins[0]
    return repr(ap)


def _dedupe_ldweights(nc):
    """Drop an InstLdweights whose weights AP is identical to the
    immediately-preceding one (only InstMatmult in between): the a/b column
    halves of mm2 share one stationary operand, and a duplicate 256-col
    DoubleRow weight load would make the weight port the bottleneck. The
    dropped load's waits move to the surviving one (deduplicated)."""
    for fn in nc.m.functions:
        for blk in fn.blocks:
            keep = []
            last_ldw = None
            removed_any = False
            for inst in blk.instructions:
                if isinstance(inst, mybir.InstLdweights):
                    if (last_ldw is not None
                            and _ldw_key(inst) == _ldw_key(last_ldw[0])
                            and inst.perf_mode == last_ldw[0].perf_mode):
                        # merge waits into the kept LDW
                        si = getattr(inst, "sync_info", None)
                        if si is not None and si.on_wait:
                            ksi = last_ldw[0].sync_info
                            if ksi is None:
                                last_ldw[0].sync_info = si
                            else:
                                have = {repr(w) for w in ksi.on_wait}
                                for w_ in si.on_wait:
                                    if repr(w_) not in have:
                                        ksi.on_wait.append(w_)
                            assert not (si.on_update or []), (
                                "dropped LDW had sem updates")
                        removed_any = True
                        continue
                    last_ldw = (inst,)
                    keep.append(inst)
                    continue
                if not isinstance(inst, mybir.InstMatmult):
                    last_ldw = None
                keep.append(inst)
            if removed_any:
                blk.instructions[:] = keep


def _strip_same_engine_waits(nc):
    """Drop redundant same-engine semaphore waits on ACT/PE compute
    instructions (each engine executes its queue in order, and TRN2 allows
    only one wait per instruction before EventSemaphore splitting)."""
    prefixes = {
        "EngineType.Activation": "Activation_",
        "EngineType.PE": "PE_",
    }
    kinds = (mybir.InstActivation, mybir.InstMatmult, mybir.InstLdweights)
    for fn in nc.m.functions:
        for blk in fn.blocks:
            for inst in blk.instructions:
                si = getattr(inst, "sync_info", None)
                if si is None or not si.on_wait or not isinstance(inst, kinds):
                    continue
                pref = prefixes.get(str(getattr(inst, "engine", None)))
                if pref is None:
                    continue
                kept = [w for w in si.on_wait
                        if not str(getattr(w, "ant_name", "")).startswith(pref)]
                if len(kept) != len(si.on_wait):
                    si.on_wait = kept


def _get_program():
    if "nc" not in _CACHE:
        _CACHE["nc"] = _build_program()
    return _CACHE["nc"]


def _make_in_maps(mk, mv, qk):
    f8 = ml_dtypes.float8_e4m3
    mkf = np.ascontiguousarray(mk.reshape(B, CK, N))
    mvf = np.ascontiguousarray(mv.reshape(B, CV, N))
    qkf = np.ascontiguousarray(qk.reshape(B, CK, P))
    in_maps = []
    for core in range(8):
        b, half = core // 2, core % 2
        n0, n1 = half * NHALF, (half + 1) * NHALF
        mk_c = mkf[b, :, n0:n1].astype(f8)             # [64, 6480]
        mk_t = np.zeros((128, NT, 128), dtype=f8)
        mk_t[:CK].reshape(CK, NT * 128)[:, :NHALF] = mk_c
        qk_c = np.zeros((128, 2048), dtype=f8)
        qk_c[:CK, :P] = qkf[b].astype(f8)
        # mvT with the ones column at 512; zeros elsewhere (incl. pad rows
        # and pad tile NT..NTP so the DoubleRow partner contributes nothing)
        mvt = np.zeros((NTP * 128, MVW), dtype=f8)
        mvt[:NHALF, :CV] = mvf[b, :, n0:n1].T.astype(f8)
        mvt[:NHALF, CV] = 1.0
        mvt_c = np.ascontiguousarray(
            mvt.reshape(NTP, 128, MVW).transpose(1, 0, 2))
        in_maps.append({"mk": np.ascontiguousarray(mk_t),
                        "qk": np.ascontiguousarray(qk_c),
                        "mvT": mvt_c})
    return in_maps


def _run(mk, mv, qk, qv, trace=False, **spmd_kwargs):
    nc = _get_program()
    in_maps = _make_in_maps(mk, mv, qk)
    res = run_bass_kernel_spmd(nc, in_maps, list(range(8)), trace=trace,
                               **spmd_kwargs)
    out = np.empty((B, 2 * CV, P), dtype=np.float32)
    for b in range(B):
        m0 = res.results[2 * b]["memT"]
        m1 = res.results[2 * b + 1]["memT"]
        ms = m0 + m1
        out[b, :CV] = (ms[:, :CV] / ms[:, CV][:, None]).T
        out[b, CV:] = qv[b].reshape(CV, P)
    return out.reshape(B, 2 * CV, H, W), res


def kernel(mk, mv, qk, qv):
    out, _ = _run(np.asarray(mk), np.asarray(mv), np.asarray(qk),
                  np.asarray(qv))
    return out


# revision 27
# speedup vs baseline: 1.0069x; 1.0069x over previous
"""Trainium2 Bass kernel for nn_MemoryReader.

Reference computation (per batch b):
    mi = mk.reshape(CK, N);  qi = qk.reshape(CK, P) / sqrt(CK)
    S  = mi.T @ qi                      # [N, P] affinity logits
    A  = softmax(S, axis=0)             # over memory axis N
    mem = mv.reshape(CV, N) @ A         # [CV, P]
    out = concat([mem, qv], axis=channel)

Sharding: 8 cores = (4 batches) x (2 halves of the memory axis N).
Each core computes, for its (b, half):
    E      = exp(0.125*S - 2)                   # fp8e4; the -2 bias keeps
                                                # E<=54 (TRN fp8e4 max 240)
                                                # and cancels in the softmax
    memT   = E.T @ [mvT | 1]                    # [P, 513]: col 512 = sum(E)
The host combines: mem = (num_0 + num_1) / (den_0 + den_1), then concats
qv (pure passthrough). No on-device collectives needed.

Key speed features vs the bf16 version:
  - mm2 runs in fp8e4 DoubleRow mode: each matmul contracts TWO 128-row
    n-tiles (lhsT/rhs get [K, 2, M] APs), ~2x column throughput.
  - The softmax denominator is a 513th "ones" column of mvT, accumulated
    by the same mm2 matmuls (split 256+257 to satisfy the one-PSUM-bank
    rule, sharing one LDWEIGHTS via a dedupe pass) - no vector-engine
    accumulation at all.
  - exp() is fused 4 n-tiles per ACT instruction (PSUM "squad" tiles) to
    amortize the ~185ns per-instruction access latency.
"""

import numpy as np
import ml_dtypes

import concourse.tile as tile
from concourse import bacc, mybir
from concourse.bass_utils import run_bass_kernel_spmd

# Problem shape (hardcoded per contract)
B, CK, CV, T, H, W = 4, 64, 512, 8, 30, 54
N = T * H * W          # 12960 memory positions
P = H * W              # 1620 query positions
NHALF = N // 2         # 6480 per core
NT = (NHALF + 127) // 128   # 51 n-tiles (last has 80 rows)
NLAST = NHALF - (NT - 1) * 128  # 80
NTP = NT + 1           # pad to even tile count for DoubleRow pairing
MVW = 528              # mvT free width: 512 mv + 1 ones + 15 pad (16B align)
CVA = 513              # real mm2 output width (512 mv + 1 denominator)
ASPL = 256             # a-half columns (b-half = 257); each fits a PSUM bank
EXP_BIAS = -2.0        # exp(0.125*s - 2): range safety for fp8e4

# p-axis chunks of 256 (2 slices of 128 each; 84-wide remainder last — it
# has the smallest output-DMA tail. Running it first was tried and is 8%
# WORSE: its mm2 consumes one mvT tile per ~330ns, 2x what the DMA queues
# can sustain while the full mvT load is still in flight.)
PCH = [(0, 256), (256, 256), (512, 256), (768, 256), (1024, 256),
       (1280, 256), (1536, 84)]
QUADS = [(0, 4), (4, 4), (8, 4), (12, 4), (16, 4), (20, 4), (24, 4), (28, 4),
         (32, 4), (36, 4), (40, 4), (44, 4), (48, 3)]

DEDUPE_LDW = True

_CACHE = {}


def _build_program():
    f8 = mybir.dt.float8e4
    bf16 = mybir.dt.bfloat16
    f32 = mybir.dt.float32
    DR = mybir.MatmulPerfMode.DoubleRow
    nc = bacc.Bacc(None, target_bir_lowering=False, debug=False)

    # mk/qk zero-padded to K=128 on the host: full-row LDWEIGHTS go through
    # the background weight buffer (K=64 loads serialize on the weight port).
    # fp8 halves the DMA feed and doubles mm1's FWL weight-load rate; the
    # logit quantization error (~0.05 absolute) is well inside the margin.
    mk_d = nc.declare_dram_parameter("mk", [128, NT, 128], f8, isOutput=False)
    # padded to 2048 so the one qk transfer has 2KB per-partition rows
    # (sub-2KB rows run ~3x slower on the DMA path)
    qk_d = nc.declare_dram_parameter("qk", [128, 2048], f8, isOutput=False)
    mvt_d = nc.declare_dram_parameter("mvT", [128, NTP, MVW], f8, isOutput=False)
    mem_d = nc.declare_dram_parameter("memT", [P, CVA], f32, isOutput=True)

    with tile.TileContext(nc) as tc:
        with (
            tc.tile_pool(name="singles", bufs=1) as singles,
            tc.tile_pool(name="epool", bufs=3) as epool,
            tc.tile_pool(name="opool", bufs=4) as opool,
            tc.tile_pool(name="spsum", bufs=2, space="PSUM") as spsum,
            tc.tile_pool(name="apsum", bufs=2, space="PSUM") as apsum,
            tc.tile_pool(name="bpsum", bufs=2, space="PSUM") as bpsum,
        ):
            qk_sb = singles.tile([128, 2048], f8)
            mk_sb = singles.tile([128, NT, 128], f8)
            mvt_sb = singles.tile([128, NTP, MVW], f8)
            # DMA facts (measured): every transfer stripes over all 16
            # queues and completes in ISSUE ORDER at aggregate BW, but
            # sub-2KB per-partition rows run ~5x slower (per-row descriptor
            # overhead), and each trigger costs ~660ns on the Sync queue.
            # So: few, fat, consumption-ordered transfers.
            nc.sync.dma_start(out=qk_sb[:, :], in_=qk_d[:, :])
            mk_groups = [(0, 16), (16, 32), (32, NT)]
            mv_groups = [(g, min(g + 4, NTP)) for g in range(0, NTP, 4)]
            order = [("mk", 0), ("mv", 0), ("mv", 1), ("mv", 2), ("mk", 1),
                     ("mv", 3), ("mv", 4), ("mv", 5), ("mv", 6), ("mk", 2),
                     ("mv", 7), ("mv", 8), ("mv", 9), ("mv", 10), ("mv", 11),
                     ("mv", 12)]
            for kind, gi in order:
                if kind == "mk":
                    g0, g1 = mk_groups[gi]
                    nc.sync.dma_start(out=mk_sb[:, g0:g1, :],
                                      in_=mk_d[:, g0:g1, :])
                else:
                    g0, g1 = mv_groups[gi]
                    nc.sync.dma_start(out=mvt_sb[:, g0:g1, :],
                                      in_=mvt_d[:, g0:g1, :])

            # Warm-up: full-size matmuls on a memset tile, depending on no
            # DMA. They fill the initial PE idle gap AND release the HAM
            # clock throttle (~3.4us of sustained activity needed).
            warmw = singles.tile([128, 128], bf16, name="warmw")
            nc.vector.memset(warmw, 1.0)
            bias_sb = singles.tile([128, 1], f32, name="bias")
            nc.vector.memset(bias_sb, EXP_BIAS)
            # The DMA path has a ~6us wake-up dead zone after the first
            # trigger (measured: first bytes land ~8.5us in), so warm
            # matmuls bridge until real inputs arrive (~32 run cold at
            # 107ns, the rest warm at 53ns -> ends ~8.5us).
            warm = spsum.tile([128, 128], f32, tag="s", name="warm")
            for _ in range(64):
                nc.tensor.matmul(warm, lhsT=warmw, rhs=warmw,
                                 start=True, stop=True)

            # Flat software pipeline over (chunk, quad) units: issue unit
            # u+1's mm1 before unit u's mm2 so the PE queue never stalls on
            # the ACT->mm2 dependency.
            units = [(ci, qi) for ci in range(len(PCH)) for qi in range(len(QUADS))]
            squads = {}
            e4s = {}
            accs = {}

            def issue_mm1(u):
                ci, qi = units[u]
                ps, w = PCH[ci]
                q0, qn = QUADS[qi]
                s = spsum.tile([128, 4, ASPL], f32, tag="s", name="s")
                for j in range(qn):
                    nt = q0 + j
                    nsz = 128 if nt < NT - 1 else NLAST
                    nc.tensor.matmul(
                        s[:nsz, j, :w],
                        lhsT=mk_sb[:, nt, :nsz],
                        rhs=qk_sb[:, ps:ps + w],
                        start=True,
                        stop=True,
                    )
                squads[u] = s

            def issue_act(u):
                ci, qi = units[u]
                ps, w = PCH[ci]
                q0, qn = QUADS[qi]
                s = squads.pop(u)
                e4 = epool.tile([128, 4, ASPL], f8, tag="e", name="e")
                # rows 80:128 of the tri-quad's last tile hold stale PSUM;
                # exp of garbage lands in e4 rows the matmuls never read.
                nc.scalar.activation(
                    out=e4[:, 0:qn, :w],
                    in_=s[:, 0:qn, :w],
                    func=mybir.ActivationFunctionType.Exp,
                    scale=0.125,  # 1/sqrt(CK)
                    bias=bias_sb[:, :],
                )
                e4s[u] = e4

            def issue_mm2(u):
                ci, qi = units[u]
                ps, w = PCH[ci]
                q0, qn = QUADS[qi]
                e4 = e4s.pop(u)
                nslices = (w + 127) // 128
                if qi == 0:
                    accs[ci] = [
                        (apsum.tile([128, 512], f32, tag="acc_a", name="acc_a"),
                         bpsum.tile([128, 512], f32, tag="acc_b", name="acc_b"))
                        for _ in range(nslices)
                    ]
                first = qi == 0
                last = qi == len(QUADS) - 1
                # DoubleRow pairs (and the odd single tile 50 at quad end)
                steps = []
                if qn == 4:
                    steps = [(0, True), (2, True)]
                else:
                    steps = [(0, True), (2, False)]
                for si, (j, dr) in enumerate(steps):
                    nt = q0 + j
                    st = first and si == 0
                    sp = last and si == len(steps) - 1
                    nsz = 128 if dr else NLAST
                    for sl in range(nslices):
                        pw = min(128, w - 128 * sl)
                        acc_a, acc_b = accs[ci][sl]
                        if dr:
                            el = e4[:nsz, j:j + 2, sl * 128:sl * 128 + pw]
                            nc.tensor.matmul(
                                acc_a[:pw, 0:ASPL],
                                lhsT=el,
                                rhs=mvt_sb[:nsz, nt:nt + 2, 0:ASPL],
                                start=st, stop=sp,
                                perf_mode=DR,
                            )
                            nc.tensor.matmul(
                                acc_b[:pw, 0:CVA - ASPL],
                                lhsT=el,
                                rhs=mvt_sb[:nsz, nt:nt + 2, ASPL:CVA],
                                start=st, stop=sp,
                                perf_mode=DR,
                            )
                        else:
                            el = e4[:nsz, j, sl * 128:sl * 128 + pw]
                            nc.tensor.matmul(
                                acc_a[:pw, 0:ASPL],
                                lhsT=el,
                                rhs=mvt_sb[:nsz, nt, 0:ASPL],
                                start=st, stop=sp,
                            )
                            nc.tensor.matmul(
                                acc_b[:pw, 0:CVA - ASPL],
                                lhsT=el,
                                rhs=mvt_sb[:nsz, nt, ASPL:CVA],
                                start=st, stop=sp,
                            )
                if last:
                    final = ci == len(PCH) - 1
                    for sl in range(nslices):
                        pw = min(128, w - 128 * sl)
                        acc_a, acc_b = accs[ci][sl]
                        o_sb = opool.tile([128, CVA], f32, tag="o", name="o")
                        nc.vector.tensor_copy(out=o_sb[:pw, 0:ASPL],
                                              in_=acc_a[:pw, 0:ASPL])
                        if final:
                            # kernel tail only: ACT is idle then, so the
                            # b-copy runs there in parallel. (Mid-stream
                            # ACT copies delay the next chunk's exp and
                            # cost ~400ns per boundary - measured.)
                            nc.scalar.activation(
                                out=o_sb[:pw, ASPL:CVA],
                                in_=acc_b[:pw, 0:CVA - ASPL],
                                func=mybir.ActivationFunctionType.Copy,
                            )
                        else:
                            nc.vector.tensor_copy(out=o_sb[:pw, ASPL:CVA],
                                                  in_=acc_b[:pw, 0:CVA - ASPL])
                        p0 = ps + sl * 128
                        nc.sync.dma_start(out=mem_d[p0:p0 + pw, :],
                                          in_=o_sb[:pw, :])
                    del accs[ci]

            issue_mm1(0)
            for u in range(len(units)):
                if u + 1 < len(units):
                    issue_mm1(u + 1)
                issue_act(u)
                issue_mm2(u)

    _strip_same_engine_waits(nc)
    if DEDUPE_LDW:
        _dedupe_ldweights(nc)
    nc.compile()
    return nc


def _ldw_key(inst):
    ap = inst.ins[0]
    return repr(ap)


def _dedupe_ldweights(nc):
    """Drop an InstLdweights whose weights AP is identical to the
    immediately-preceding one (only InstMatmult in between): the a/b column
    halves of mm2 share one stationary operand, and a duplicate 256-col
    DoubleRow weight load would make the weight port the bottleneck. The
    dropped load's waits move to the surviving one (deduplicated)."""
    for fn in nc.m.functions:
        for blk in fn.blocks:
            keep = []
            last_ldw = None
            removed_any = False
            for inst in blk.instructions:
                if isinstance(inst, mybir.InstLdweights):
                    if (last_ldw is not None
                            and _ldw_key(inst) == _ldw_key(last_ldw[0])
                            and inst.perf_mode == last_ldw[0].perf_mode):
                        # merge waits into the kept LDW
                        si = getattr(inst, "sync_info", None)
                        if si is not None and si.on_wait:
                            ksi = last_ldw[0].sync_info
                            if ksi is None:
                                last_ldw[0].sync_info = si
                            else:
                                have = {repr(w) for w in ksi.on_wait}
                                for w_ in si.on_wait:
                                    if repr(w_) not in have:
                                        ksi.on_wait.append(w_)
                            assert not (si.on_update or []), (
                                "dropped LDW had sem updates")
                        removed_any = True
                        continue
                    last_ldw = (inst,)
                    keep.append(inst)
                    continue
                if not isinstance(inst, mybir.InstMatmult):
                    last_ldw = None
                keep.append(inst)
            if removed_any:
                blk.instructions[:] = keep


def _strip_same_engine_waits(nc):
    """Drop redundant same-engine semaphore waits on ACT/PE compute
    instructions (each engine executes its queue in order, and TRN2 allows
    only one wait per instruction before EventSemaphore splitting)."""
    prefixes = {
        "EngineType.Activation": "Activation_",
        "EngineType.PE": "PE_",
    }
    kinds = (mybir.InstActivation, mybir.InstMatmult, mybir.InstLdweights)
    for fn in nc.m.functions:
        for blk in fn.blocks:
            for inst in blk.instructions:
                si = getattr(inst, "sync_info", None)
                if si is None or not si.on_wait or not isinstance(inst, kinds):
                    continue
                pref = prefixes.get(str(getattr(inst, "engine", None)))
                if pref is None:
                    continue
                kept = [w for w in si.on_wait
                        if not str(getattr(w, "ant_name", "")).startswith(pref)]
                if len(kept) != len(si.on_wait):
                    si.on_wait = kept


def _get_program():
    if "nc" not in _CACHE:
        _CACHE["nc"] = _build_program()
    return _CACHE["nc"]


def _make_in_maps(mk, mv, qk):
    f8 = ml_dtypes.float8_e4m3
    mkf = np.ascontiguousarray(mk.reshape(B, CK, N))
    mvf = np.ascontiguousarray(mv.reshape(B, CV, N))
    qkf = np.ascontiguousarray(qk.reshape(B, CK, P))
    in_maps = []
    for core in range(8):
        b, half = core // 2, core % 2
        n0, n1 = half * NHALF, (half + 1) * NHALF
        mk_c = mkf[b, :, n0:n1].astype(f8)             # [64, 6480]
        mk_t = np.zeros((128, NT, 128), dtype=f8)
        mk_t[:CK].reshape(CK, NT * 128)[:, :NHALF] = mk_c
        qk_c = np.zeros((128, 2048), dtype=f8)
        qk_c[:CK, :P] = qkf[b].astype(f8)
        # mvT with the ones column at 512; zeros elsewhere (incl. pad rows
        # and pad tile NT..NTP so the DoubleRow partner contributes nothing)
        mvt = np.zeros((NTP * 128, MVW), dtype=f8)
        mvt[:NHALF, :CV] = mvf[b, :, n0:n1].T.astype(f8)
        mvt[:NHALF, CV] = 1.0
        mvt_c = np.ascontiguousarray(
            mvt.reshape(NTP, 128, MVW).transpose(1, 0, 2))
        in_maps.append({"mk": np.ascontiguousarray(mk_t),
                        "qk": np.ascontiguousarray(qk_c),
                        "mvT": mvt_c})
    return in_maps


def _run(mk, mv, qk, qv, trace=False, **spmd_kwargs):
    nc = _get_program()
    in_maps = _make_in_maps(mk, mv, qk)
    res = run_bass_kernel_spmd(nc, in_maps, list(range(8)), trace=trace,
                               **spmd_kwargs)
    out = np.empty((B, 2 * CV, P), dtype=np.float32)
    for b in range(B):
        m0 = res.results[2 * b]["memT"]
        m1 = res.results[2 * b + 1]["memT"]
        ms = m0 + m1
        out[b, :CV] = (ms[:, :CV] / ms[:, CV][:, None]).T
        out[b, CV:] = qv[b].reshape(CV, P)
    return out.reshape(B, 2 * CV, H, W), res


def kernel(mk, mv, qk, qv):
    out, _ = _run(np.asarray(mk), np.asarray(mv), np.asarray(qk),
                  np.asarray(qv))
    return out
